# revision 1
# baseline (speedup 1.0000x reference)
"""Trainium2 Bass kernel for nn_CopiedSetEncoder (set encoder with recurrent
attention). Self-contained: shards batch across 8 NeuronCores, builds a
length-specialized SPMD Tile kernel, runs it, and reassembles the output.
"""
import os

import numpy as np

import concourse.bass as bass
import concourse.mybir as mybir
import concourse.tile as tile
from concourse.bass_utils import run_bass_kernel_spmd

B, F_, D_IN = 128, 1024, 128
H1, H2, E, H = 512, 512, 256, 256
N_SHUFFLE = 5
NCORES = 8
BLOC = B // NCORES  # 16 batches per core
NEG = -1e30
C1 = 15.0  # logit shift for max-free softmax

f32 = mybir.dt.float32
f16 = mybir.dt.float16


def _split_multi_waits(nc):
    """HW allows at most one sync wait per instruction; hoist extras into
    standalone InstEventSemaphore carriers on the same engine."""
    cnt = 0
    for bb in nc.main_func.blocks:
        insts = bb.instructions  # live list
        i = 0
        while i < len(insts):
            ins = insts[i]
            si = ins.sync_info
            if si is not None and si.on_wait and len(si.on_wait) > 1:
                waits = list(si.on_wait)
                carriers = []
                for w in waits[:-1]:
                    cnt += 1
                    ev = mybir.InstEventSemaphore(name=f"wsplit-{cnt}")
                    ev.engine = ins.engine
                    ev.sync_info = mybir.SyncInfo(on_wait=[w], on_update=[])
                    carriers.append(ev)
                ins.sync_info = mybir.SyncInfo(
                    on_wait=[waits[-1]], on_update=list(si.on_update)
                )
                for j, ev in enumerate(carriers):
                    insts.insert(i + j, ev)
                    nc.register_instruction(ev, overwrite=True)
                i += len(carriers)
            i += 1
    return cnt


PHASE = os.environ.get("KER_PHASE", "full")


def _build_module(n_chunks, t_common):
    """One SPMD program for all cores. n_chunks[j] = 128-token chunks for
    batch slot j (uniform across cores); t_common = sum(n_chunks)*128."""
    nc = bass.Bass()
    n_tiles = t_common // 512
    off = np.concatenate(([0], np.cumsum(np.asarray(n_chunks) * 128)))
    tot_chunks = t_common // 128
    max_nc = max(n_chunks)

    # ---- inputs ----
    xT_e = nc.declare_dram_parameter("xT", [128, t_common], f16, isOutput=False)
    w1_e = nc.declare_dram_parameter("w1", [128, H1], f16, isOutput=False)
    w2_e = nc.declare_dram_parameter("w2", [128, 4, H2], f16, isOutput=False)
    w3_e = nc.declare_dram_parameter("w3", [128, 4, E], f16, isOutput=False)
    wih_e = nc.declare_dram_parameter("wih", [128, 2, 4 * H], f16, isOutput=False)
    whh_e = nc.declare_dram_parameter("whh", [128, 2, 4 * H], f16, isOutput=False)
    b1_e = nc.declare_dram_parameter("b1", [128, 4], f32, isOutput=False)
    b2_e = nc.declare_dram_parameter("b2", [128, 4], f32, isOutput=False)
    bg_e = nc.declare_dram_parameter("bg", [128, 8], f32, isOutput=False)
    mask_e = nc.declare_dram_parameter(
        "mask", [128, tot_chunks, BLOC], f32, isOutput=False
    )
    w0T_e = nc.declare_dram_parameter(
        "w0T", [128, tot_chunks, BLOC], f16, isOutput=False
    )
    ones1_e = nc.declare_dram_parameter("ones1", [1, 128], f32, isOutput=False)
    ident_e = nc.declare_dram_parameter("ident", [128, 128], f32, isOutput=False)
    att_o = nc.declare_dram_parameter("att", [BLOC, E], f32, isOutput=True)
    qt_o = nc.declare_dram_parameter("qt", [BLOC, H], f32, isOutput=True)

    with tile.TileContext(nc) as tc:
        with tc.tile_pool(name="big", bufs=1) as big, \
             tc.tile_pool(name="wp", bufs=1) as wp:
            # resident tensors
            xT = big.tile([128, t_common], f16)
            embA = big.tile([128, 2, t_common], f16)
            embB = big.tile([128, tot_chunks, E], f16)
            w1 = wp.tile([128, H1], f16)
            w2 = wp.tile([128, 4, H2], f16)
            w3 = wp.tile([128, 4, E], f16)
            wih = wp.tile([128, 2, 4 * H], f16)
            whh = wp.tile([128, 2, 4 * H], f16)
            b1 = wp.tile([128, 4], f32)
            b2 = wp.tile([128, 4], f32)
            bg = wp.tile([128, 8], f32)
            mask = wp.tile([128, tot_chunks, BLOC], f32)
            w0T = wp.tile([128, tot_chunks, BLOC], f16)
            ones1 = wp.tile([1, 128], f32)
            ident = wp.tile([128, 128], f32)
            for dst, src in [
                (xT, xT_e), (w1, w1_e), (w2, w2_e), (w3, w3_e),
                (wih, wih_e), (whh, whh_e), (b1, b1_e), (b2, b2_e),
                (bg, bg_e), (mask, mask_e), (w0T, w0T_e), (ident, ident_e),
                (ones1, ones1_e),
            ]:
                nc.sync.dma_start(out=dst[:], in_=src[:])

            # ---- phase 1: MLP over 512-token tiles ----
            with tc.tile_pool(name="mlp", bufs=3) as mp, \
                 tc.tile_pool(name="ps1", bufs=2, space="PSUM") as ps1, \
                 tc.tile_pool(name="ps2", bufs=2, space="PSUM") as ps2, \
                 tc.tile_pool(name="ps3", bufs=2, space="PSUM") as ps3:
                for t in range(n_tiles):
                    sl = slice(t * 512, (t + 1) * 512)
                    h1t = mp.tile([128, 4, 512], f16, tag="h1")
                    for mc in range(4):
                        p = ps1.tile([128, 512], f32, tag="pA")
                        nc.tensor.matmul(
                            p[:], w1[:, mc * 128:(mc + 1) * 128], xT[:, sl],
                            start=True, stop=True,
                        )
                        if mc % 2 == 0:
                            nc.scalar.activation(
                                out=h1t[:, mc, :], in_=p[:],
                                func=mybir.ActivationFunctionType.Relu,
                                bias=b1[:, mc:mc + 1], scale=1.0,
                            )
                        else:
                            nc.vector.tensor_scalar(
                                out=h1t[:, mc, :], in0=p[:], scalar1=b1[:, mc:mc + 1],
                                scalar2=0.0, op0=mybir.AluOpType.add,
                                op1=mybir.AluOpType.max,
                            )
                    h2t = mp.tile([128, 4, 512], f16, tag="h2")
                    for mc in range(4):
                        p = ps2.tile([128, 512], f32, tag="pB")
                        for kc in range(4):
                            nc.tensor.matmul(
                                p[:], w2[:, kc, mc * 128:(mc + 1) * 128],
                                h1t[:, kc, :], start=(kc == 0), stop=(kc == 3),
                            )
                        if mc % 2 == 0:
                            nc.scalar.activation(
                                out=h2t[:, mc, :], in_=p[:],
                                func=mybir.ActivationFunctionType.Relu,
                                bias=b2[:, mc:mc + 1], scale=1.0,
                            )
                        else:
                            nc.vector.tensor_scalar(
                                out=h2t[:, mc, :], in0=p[:], scalar1=b2[:, mc:mc + 1],
                                scalar2=0.0, op0=mybir.AluOpType.add,
                                op1=mybir.AluOpType.max,
                            )
                    # embA: [e-chunk partitions, tokens]
                    for mc in range(2):
                        p = ps3.tile([128, 512], f32, tag="pC")
                        for kc in range(4):
                            nc.tensor.matmul(
                                p[:], w3[:, kc, mc * 128:(mc + 1) * 128],
                                h2t[:, kc, :], start=(kc == 0), stop=(kc == 3),
                            )
                        nc.scalar.copy(out=embA[:, mc, sl], in_=p[:])
                    # embB: [token partitions, e] via h2-stationary matmuls
                    for s in range(4):
                        p = ps3.tile([128, 256], f32, tag="pD")
                        tsl = slice(s * 128, (s + 1) * 128)
                        for kc in range(4):
                            nc.tensor.matmul(
                                p[:], h2t[:, kc, tsl], w3[:, kc, :],
                                start=(kc == 0), stop=(kc == 3),
                            )
                        nc.vector.tensor_copy(embB[:, t * 4 + s, :], p[:])

            if PHASE == "mlp":
                with tc.tile_pool(name="dummy", bufs=1) as dp:
                    da = dp.tile([BLOC, E], f32)
                    dq = dp.tile([BLOC, H], f32)
                    nc.vector.tensor_copy(da[:], embA[:BLOC, 0, :E])
                    nc.vector.tensor_copy(dq[:], embB[:BLOC, 0, :])
                    nc.sync.dma_start(out=att_o[:], in_=da[:])
                    nc.sync.dma_start(out=qt_o[:], in_=dq[:])

            # ---- phase 2: recurrent attention ----
            if PHASE != "mlp":
              with tc.tile_pool(name="att", bufs=1) as ap, \
                 tc.tile_pool(name="attd", bufs=2) as ad, \
                 tc.tile_pool(name="psL", bufs=2, space="PSUM") as psL, \
                 tc.tile_pool(name="psA", bufs=1, space="PSUM") as psA, \
                 tc.tile_pool(name="psG", bufs=1, space="PSUM") as psG, \
                 tc.tile_pool(name="psT", bufs=1, space="PSUM") as psT:
                qtT = ap.tile([128, 2, BLOC], f16)      # query, [h, b]
                qtT32 = ap.tile([128, 2, BLOC], f32)
                ct = ap.tile([128, 2, BLOC], f32)       # cell state
                att_sb = ap.tile([BLOC, E], f32)
                attT = ap.tile([128, 2, BLOC], f16)
                lgT = ap.tile([128, tot_chunks, BLOC], f32)  # shifted logits
                onesc = ap.tile([128, 1], f32)
                wTn = ap.tile([128, tot_chunks, BLOC], f16)  # normalized weights
                nc.vector.memset(qtT[:], 0.0)
                nc.vector.memset(ct[:], 0.0)
                nc.vector.memset(onesc[:], 1.0)

                n_grp = (tot_chunks + 7) // 8
                for it in range(N_SHUFFLE):
                    if it > 0:
                        # logits token-major: chunk-stationary matmuls
                        for g in range(n_grp):
                            nch = min(8, tot_chunks - g * 8)
                            lgp = psL.tile([128, 8, BLOC], f32, tag="lgp")
                            for ci in range(nch):
                                c = g * 8 + ci
                                for kc in range(2):
                                    nc.tensor.matmul(
                                        lgp[:, ci, :],
                                        embA[:, kc, c * 128:(c + 1) * 128],
                                        qtT[:, kc, :],
                                        start=(kc == 0), stop=(kc == 1),
                                    )
                            # shift+mask into sbuf (f32)
                            nc.vector.tensor_tensor(
                                out=lgT[:, g * 8: g * 8 + nch, :],
                                in0=lgp[:, :nch, :],
                                in1=mask[:, g * 8: g * 8 + nch, :],
                                op=mybir.AluOpType.add,
                            )
                        # exp (pass 1, fp32, pre-shifted by mask)
                        w1T = ad.tile([128, tot_chunks, BLOC], f32, tag="w1T")
                        nc.scalar.activation(
                            out=w1T[:], in_=lgT[:],
                            func=mybir.ActivationFunctionType.Exp,
                        )
                        # S_j = sum over tokens: fp32 chunk matmuls vs ones
                        s_ps = psT.tile([BLOC, 1], f32, tag="t1")
                        for c in range(tot_chunks):
                            nc.tensor.matmul(
                                s_ps[:, :], w1T[:, c, :], onesc[:],
                                start=(c == 0), stop=(c == tot_chunks - 1),
                            )
                        rS = ad.tile([BLOC, 1], f32, tag="rS")
                        nc.vector.reciprocal(rS[:], s_ps[:])
                        # broadcast 1/S to [128, BLOC]: transpose + K=1 matmul
                        rT_ps = psT.tile([1, BLOC], f32, tag="t1")
                        nc.tensor.transpose(rT_ps[:], rS[:], ident[:BLOC, :BLOC])
                        rRow = ad.tile([1, BLOC], f32, tag="rRow")
                        nc.vector.tensor_copy(rRow[:], rT_ps[:])
                        rB_ps = psT.tile([128, BLOC], f32, tag="t1")
                        nc.tensor.matmul(
                            rB_ps[:], ones1[:], rRow[:], start=True, stop=True
                        )
                        rB = ad.tile([128, BLOC], f32, tag="rB2")
                        nc.vector.tensor_copy(rB[:], rB_ps[:])
                        # normalized fp16 weights: w1T * (1/S) broadcast
                        rb_ap = rB[:]
                        rB_b = bass.AP(
                            tensor=rb_ap.tensor, offset=rb_ap.offset,
                            ap=[list(rb_ap.ap[0]), [0, tot_chunks],
                                list(rb_ap.ap[1])],
                        )
                        nc.vector.tensor_tensor(
                            out=wTn[:], in0=w1T[:], in1=rB_b,
                            op=mybir.AluOpType.mult,
                        )
                        wsrc = wTn
                    else:
                        wsrc = w0T

                    # attended: accumulate all chunks, M=16
                    att_ps = psA.tile([BLOC, E], f32)
                    for c in range(tot_chunks):
                        nc.tensor.matmul(
                            att_ps[:, :], wsrc[:, c, :], embB[:, c, :],
                            start=(c == 0), stop=(c == tot_chunks - 1),
                        )
                    nc.vector.tensor_copy(att_sb[:], att_ps[:])
                    # attT: [16, 256] -> [128, 2, 16]
                    for c in range(2):
                        pt = psT.tile([128, BLOC], f32, tag="pt")
                        nc.tensor.transpose(
                            pt[:], att_sb[:, c * 128:(c + 1) * 128],
                            ident[:BLOC, :BLOC],
                        )
                        nc.scalar.copy(out=attT[:, c, :], in_=pt[:])

                    # LSTM gates = Wih @ att + Whh @ qt + bg
                    g_ps = psG.tile([128, 8, BLOC], f32)
                    for mc in range(8):
                        msl = slice(mc * 128, (mc + 1) * 128)
                        for kc in range(2):
                            nc.tensor.matmul(
                                g_ps[:, mc, :], wih[:, kc, msl], attT[:, kc, :],
                                start=(kc == 0), stop=False,
                            )
                        for kc in range(2):
                            nc.tensor.matmul(
                                g_ps[:, mc, :], whh[:, kc, msl],
                                qtT[:, kc, :],
                                start=False, stop=(kc == 1),
                            )
                    ig = ad.tile([128, 2, BLOC], f32, tag="ig")
                    fg = ad.tile([128, 2, BLOC], f32, tag="fg")
                    gg = ad.tile([128, 2, BLOC], f32, tag="gg")
                    og = ad.tile([128, 2, BLOC], f32, tag="og")
                    for c in range(2):
                        nc.scalar.activation(
                            out=ig[:, c, :], in_=g_ps[:, c, :],
                            func=mybir.ActivationFunctionType.Sigmoid,
                            bias=bg[:, c:c + 1], scale=1.0,
                        )
                        nc.scalar.activation(
                            out=fg[:, c, :], in_=g_ps[:, 2 + c, :],
                            func=mybir.ActivationFunctionType.Sigmoid,
                            bias=bg[:, 2 + c:3 + c], scale=1.0,
                        )
                        nc.scalar.activation(
                            out=gg[:, c, :], in_=g_ps[:, 4 + c, :],
                            func=mybir.ActivationFunctionType.Tanh,
                            bias=bg[:, 4 + c:5 + c], scale=1.0,
                        )
                        nc.scalar.activation(
                            out=og[:, c, :], in_=g_ps[:, 6 + c, :],
                            func=mybir.ActivationFunctionType.Sigmoid,
                            bias=bg[:, 6 + c:7 + c], scale=1.0,
                        )
                    tmp = ad.tile([128, 2, BLOC], f32, tag="tmp")
                    nc.vector.tensor_tensor(
                        out=tmp[:], in0=ig[:], in1=gg[:], op=mybir.AluOpType.mult
                    )
                    nc.vector.tensor_tensor(
                        out=ct[:], in0=fg[:], in1=ct[:], op=mybir.AluOpType.mult
                    )
                    nc.vector.tensor_tensor(
                        out=ct[:], in0=ct[:], in1=tmp[:], op=mybir.AluOpType.add
                    )
                    th = ad.tile([128, 2, BLOC], f32, tag="th")
                    for c in range(2):
                        nc.scalar.activation(
                            out=th[:, c, :], in_=ct[:, c, :],
                            func=mybir.ActivationFunctionType.Tanh,
                        )
                    nc.vector.tensor_tensor(
                        out=qtT32[:], in0=og[:], in1=th[:], op=mybir.AluOpType.mult
                    )
                    nc.vector.tensor_copy(qtT[:], qtT32[:])

                # outputs
                nc.sync.dma_start(out=att_o[:], in_=att_sb[:])
                qt_out = ap.tile([BLOC, H], f32)
                for c in range(2):
                    pt = psT.tile([BLOC, 128], f32, tag="ptq")
                    nc.tensor.transpose(pt[:], qtT32[:, c, :], ident[:, :])
                    nc.vector.tensor_copy(qt_out[:, c * 128:(c + 1) * 128], pt[:])
                nc.sync.dma_start(out=qt_o[:], in_=qt_out[:])

    _split_multi_waits(nc)
    return nc


def kernel(state, length, W1, b1, W2, b2, W3, b3, W_ih, W_hh, b_ih, b_hh):
    state = np.asarray(state, dtype=np.float32)
    length = np.asarray(length, dtype=np.int32)
    lengths = length.astype(np.int64)

    # sorted snake assignment: slot j of core c = order[j*8 + c]
    order = np.argsort(-lengths, kind="stable")
    perm = order.reshape(BLOC, NCORES)  # perm[j, c]
    lens = lengths[perm]  # [BLOC, NCORES]
    n_chunks = [int(max(1, int(np.max(np.ceil(lens[j] / 128))))) for j in range(BLOC)]
    t_common = -(-(sum(n_chunks) * 128) // 512) * 512  # pad to 512 multiple
    off = np.concatenate(([0], np.cumsum(np.asarray(n_chunks) * 128)))
    tot_chunks = t_common // 128

    nc = _build_module(n_chunks, t_common)

    # host-side weight prep (shared across cores)
    w1h = W1.T.astype(np.float16)                                # [128, 512]
    w2h = np.ascontiguousarray(
        W2.T.reshape(4, 128, H2).transpose(1, 0, 2)
    ).astype(np.float16)                                         # [128, 4, 512]
    w3h = np.ascontiguousarray(
        W3.T.reshape(4, 128, E).transpose(1, 0, 2)
    ).astype(np.float16)                                         # [128, 4, 256]
    wihh = np.ascontiguousarray(
        W_ih.T.reshape(2, 128, 4 * H).transpose(1, 0, 2)
    ).astype(np.float16)                                         # [128, 2, 1024]
    whhh = np.ascontiguousarray(
        W_hh.T.reshape(2, 128, 4 * H).transpose(1, 0, 2)
    ).astype(np.float16)
    b1h = np.ascontiguousarray(b1.reshape(4, 128).T).astype(np.float32)
    b2h = np.ascontiguousarray(b2.reshape(4, 128).T).astype(np.float32)
    bgv = (b_ih + b_hh + W_ih @ b3).astype(np.float32)
    bgh = np.ascontiguousarray(bgv.reshape(8, 128).T).astype(np.float32)
    identh = np.eye(128, dtype=np.float32)
    ones1h = np.ones((1, 128), dtype=np.float32)

    in_maps = []
    for c in range(NCORES):
        bidx = perm[:, c]  # batch index per slot
        xT = np.zeros((128, t_common), dtype=np.float16)
        maskh = np.full((128, tot_chunks, BLOC), NEG, dtype=np.float32)
        w0T = np.zeros((128, tot_chunks, BLOC), dtype=np.float16)
        for j in range(BLOC):
            ln = int(lengths[bidx[j]])
            xT[:, off[j]: off[j] + ln] = state[bidx[j], :ln, :].T
            c0 = off[j] // 128
            valid = np.zeros(n_chunks[j] * 128, dtype=bool)
            valid[:ln] = True
            vT = valid.reshape(n_chunks[j], 128).T  # [128, n_chunks_j]
            maskh[:, c0:c0 + n_chunks[j], j] = np.where(vT, -C1, NEG)
            w0T[:, c0:c0 + n_chunks[j], j] = np.where(
                vT, 1.0 / ln, 0.0
            ).astype(np.float16)
        in_maps.append({
            "xT": xT, "w1": w1h, "w2": w2h, "w3": w3h,
            "wih": wihh, "whh": whhh, "b1": b1h, "b2": b2h, "bg": bgh,
            "mask": maskh, "w0T": w0T, "ident": identh, "ones1": ones1h,
        })

    res = run_bass_kernel_spmd(nc, in_maps, list(range(NCORES)))

    out = np.zeros((B, E + H), dtype=np.float32)
    for c in range(NCORES):
        att = res.results[c]["att"] + b3[None, :].astype(np.float32)
        qt = res.results[c]["qt"]
        for j in range(BLOC):
            out[perm[j, c], :E] = att[j]
            out[perm[j, c], E:] = qt[j]
    return out



# revision 3
# speedup vs baseline: 1.1945x; 1.1945x over previous
"""Trainium2 Bass kernel for nn_CopiedSetEncoder (set encoder with recurrent
attention). Self-contained: shards batch across 8 NeuronCores, builds a
length-specialized SPMD Tile kernel, runs it, and reassembles the output.

v2: embB via PE transposes of embA (saves ~3k PE cycles/tile), softmax
denominator folded into the attended matmul via a ones-column (kills the
serial S-sum chain), post-hoc normalization, LSTM sigmoid/tanh computed
from Exp only (no activation-table thrash), gate biases folded in as K=1
matmuls, staged input DMA so tile-0 compute starts immediately.
"""
import os

import numpy as np

import concourse.bass as bass
import concourse.mybir as mybir
import concourse.tile as tile
from concourse.bass_utils import run_bass_kernel_spmd

B, F_, D_IN = 128, 1024, 128
H1, H2, E, H = 512, 512, 256, 256
N_SHUFFLE = 5
NCORES = 8
BLOC = B // NCORES  # 16 batches per core
NEG = -1e30

f32 = mybir.dt.float32
f16 = mybir.dt.float16


def _split_multi_waits(nc):
    """HW allows at most one sync wait per instruction; hoist extras into
    standalone InstEventSemaphore carriers on the same engine."""
    cnt = 0
    for bb in nc.main_func.blocks:
        insts = bb.instructions  # live list
        i = 0
        while i < len(insts):
            ins = insts[i]
            si = ins.sync_info
            if si is not None and si.on_wait and len(si.on_wait) > 1:
                waits = list(si.on_wait)
                carriers = []
                for w in waits[:-1]:
                    cnt += 1
                    ev = mybir.InstEventSemaphore(name=f"wsplit-{cnt}")
                    ev.engine = ins.engine
                    ev.sync_info = mybir.SyncInfo(on_wait=[w], on_update=[])
                    carriers.append(ev)
                ins.sync_info = mybir.SyncInfo(
                    on_wait=[waits[-1]], on_update=list(si.on_update)
                )
                for j, ev in enumerate(carriers):
                    insts.insert(i + j, ev)
                    nc.register_instruction(ev, overwrite=True)
                i += len(carriers)
            i += 1
    return cnt


PHASE = os.environ.get("KER_PHASE", "full")
Exp = mybir.ActivationFunctionType.Exp


def _build_module(n_chunks, t_common):
    """One SPMD program for all cores. n_chunks[j] = 128-token chunks for
    batch slot j (uniform across cores); t_common = sum(n_chunks)*128
    rounded up to a 512 multiple."""
    nc = bass.Bass()
    n_tiles = t_common // 512
    tot_chunks = t_common // 128
    sum_chunks = sum(n_chunks)  # live chunks (tail padding excluded)

    # ---- inputs ----
    xT_e = nc.declare_dram_parameter("xT", [128, t_common], f16, isOutput=False)
    w1_e = nc.declare_dram_parameter("w1", [128, H1], f16, isOutput=False)
    w2_e = nc.declare_dram_parameter("w2", [128, 4, H2], f16, isOutput=False)
    w3_e = nc.declare_dram_parameter("w3", [128, 4, E], f16, isOutput=False)
    wih_e = nc.declare_dram_parameter("wih", [128, 2, 4 * H], f16, isOutput=False)
    whh_e = nc.declare_dram_parameter("whh", [128, 2, 4 * H], f16, isOutput=False)
    b1_e = nc.declare_dram_parameter("b1", [128, 4], f32, isOutput=False)
    b2_e = nc.declare_dram_parameter("b2", [128, 4], f32, isOutput=False)
    bgr_e = nc.declare_dram_parameter("bgr", [1, 4 * H], f32, isOutput=False)
    mask_e = nc.declare_dram_parameter(
        "mask", [128, sum_chunks, BLOC], f32, isOutput=False
    )
    w0T_e = nc.declare_dram_parameter(
        "w0T", [128, sum_chunks, BLOC], f16, isOutput=False
    )
    idf16_e = nc.declare_dram_parameter("idf16", [128, 128], f16, isOutput=False)
    idf32_e = nc.declare_dram_parameter("idf32", [128, 128], f32, isOutput=False)
    att_o = nc.declare_dram_parameter("att", [BLOC, E], f32, isOutput=True)
    qt_o = nc.declare_dram_parameter("qt", [BLOC, H], f32, isOutput=True)

    with tile.TileContext(nc) as tc:
        with tc.tile_pool(name="big", bufs=1) as big, \
             tc.tile_pool(name="wp", bufs=1) as wp:
            # resident tensors
            xT = big.tile([128, t_common], f16)
            embA = big.tile([128, 2, t_common], f16)
            embB = big.tile([128, tot_chunks, E + 1], f16)
            w1 = wp.tile([128, H1], f16)
            w2 = wp.tile([128, 4, H2], f16)
            w3 = wp.tile([128, 4, E], f16)
            wih = wp.tile([128, 2, 4 * H], f16)
            whh = wp.tile([128, 2, 4 * H], f16)
            b1 = wp.tile([128, 4], f32)
            b2 = wp.tile([128, 4], f32)
            bgr = wp.tile([1, 4 * H], f32)
            onesB = wp.tile([1, BLOC], f32)
            mask = wp.tile([128, sum_chunks, BLOC], f32)
            w0T = wp.tile([128, sum_chunks, BLOC], f16)
            idf16 = wp.tile([128, 128], f16)
            idf32 = wp.tile([128, 128], f32)

            # staged DMA: phase-1-critical first, phase-2 tensors later
            nc.sync.dma_start(out=w1[:], in_=w1_e[:])
            nc.sync.dma_start(out=b1[:], in_=b1_e[:])
            # xT in pieces so tile-0 compute starts immediately
            xsplit = sorted({min(v, t_common) for v in (0, 512, 1024, 3072, 6144)})
            xsplit.append(t_common)
            for a, b in zip(xsplit[:-1], xsplit[1:]):
                if b > a:
                    nc.sync.dma_start(out=xT[:, a:b], in_=xT_e[:, a:b])
            nc.sync.dma_start(out=w2[:], in_=w2_e[:])
            nc.sync.dma_start(out=b2[:], in_=b2_e[:])
            nc.sync.dma_start(out=w3[:], in_=w3_e[:])
            nc.sync.dma_start(out=idf16[:], in_=idf16_e[:])
            nc.sync.dma_start(out=idf32[:], in_=idf32_e[:])
            nc.sync.dma_start(out=w0T[:], in_=w0T_e[:])
            nc.sync.dma_start(out=wih[:], in_=wih_e[:])
            nc.sync.dma_start(out=whh[:], in_=whh_e[:])
            nc.sync.dma_start(out=bgr[:], in_=bgr_e[:])
            nc.sync.dma_start(out=mask[:], in_=mask_e[:])
            nc.vector.memset(onesB[:], 1.0)
            # softmax-denominator column of embB (mask already zeroes the
            # weights of invalid tokens, so a plain 1 everywhere is exact)
            nc.vector.memset(embB[:, :, E], 1.0)

            # ---- phase 1: MLP over 512-token tiles; embB by transpose ----
            with tc.tile_pool(name="mlp", bufs=3) as mp, \
                 tc.tile_pool(name="ps1", bufs=2, space="PSUM") as ps1, \
                 tc.tile_pool(name="ps2", bufs=2, space="PSUM") as ps2, \
                 tc.tile_pool(name="ps3", bufs=2, space="PSUM") as ps3, \
                 tc.tile_pool(name="psTr", bufs=2, space="PSUM") as psTr:
                for t in range(n_tiles):
                    sl = slice(t * 512, (t + 1) * 512)
                    h1t = mp.tile([128, 4, 512], f16, tag="h1")
                    for mc in range(4):
                        p = ps1.tile([128, 512], f32, tag="pA")
                        nc.tensor.matmul(
                            p[:], w1[:, mc * 128:(mc + 1) * 128], xT[:, sl],
                            start=True, stop=True,
                        )
                        if mc % 2 == 0:
                            nc.scalar.activation(
                                out=h1t[:, mc, :], in_=p[:],
                                func=mybir.ActivationFunctionType.Relu,
                                bias=b1[:, mc:mc + 1], scale=1.0,
                            )
                        else:
                            nc.vector.tensor_scalar(
                                out=h1t[:, mc, :], in0=p[:], scalar1=b1[:, mc:mc + 1],
                                scalar2=0.0, op0=mybir.AluOpType.add,
                                op1=mybir.AluOpType.max,
                            )
                    h2t = mp.tile([128, 4, 512], f16, tag="h2")
                    for mc in range(4):
                        p = ps2.tile([128, 512], f32, tag="pB")
                        for kc in range(4):
                            nc.tensor.matmul(
                                p[:], w2[:, kc, mc * 128:(mc + 1) * 128],
                                h1t[:, kc, :], start=(kc == 0), stop=(kc == 3),
                            )
                        if mc % 2 == 0:
                            nc.scalar.activation(
                                out=h2t[:, mc, :], in_=p[:],
                                func=mybir.ActivationFunctionType.Relu,
                                bias=b2[:, mc:mc + 1], scale=1.0,
                            )
                        else:
                            nc.vector.tensor_scalar(
                                out=h2t[:, mc, :], in0=p[:], scalar1=b2[:, mc:mc + 1],
                                scalar2=0.0, op0=mybir.AluOpType.add,
                                op1=mybir.AluOpType.max,
                            )
                    # embA: [e-chunk partitions, tokens]
                    for mc in range(2):
                        p = ps3.tile([128, 512], f32, tag="pC")
                        for kc in range(4):
                            nc.tensor.matmul(
                                p[:], w3[:, kc, mc * 128:(mc + 1) * 128],
                                h2t[:, kc, :], start=(kc == 0), stop=(kc == 3),
                            )
                        nc.scalar.copy(out=embA[:, mc, sl], in_=p[:])
                    # embB: [token partitions, e] via PE transposes of embA
                    for s in range(4):
                        c = t * 4 + s
                        tsl = slice(t * 512 + s * 128, t * 512 + (s + 1) * 128)
                        for h in range(2):
                            pt = psTr.tile([128, 128], f16, tag="pT")
                            nc.tensor.transpose(pt[:], embA[:, h, tsl], idf16[:])
                            nc.vector.tensor_copy(
                                embB[:, c, h * 128:(h + 1) * 128], pt[:]
                            )

            if PHASE == "mlp":
                with tc.tile_pool(name="dummy", bufs=1) as dp:
                    da = dp.tile([BLOC, E], f32)
                    dq = dp.tile([BLOC, H], f32)
                    nc.vector.tensor_copy(da[:], embA[:BLOC, 0, :E])
                    nc.vector.tensor_copy(dq[:], embB[:BLOC, 0, :H])
                    nc.sync.dma_start(out=att_o[:], in_=da[:])
                    nc.sync.dma_start(out=qt_o[:], in_=dq[:])

            # ---- phase 2: recurrent attention ----
            if PHASE != "mlp":
              with tc.tile_pool(name="att", bufs=1) as ap, \
                 tc.tile_pool(name="attd", bufs=2) as ad, \
                 tc.tile_pool(name="psL", bufs=2, space="PSUM") as psL, \
                 tc.tile_pool(name="psA", bufs=1, space="PSUM") as psA, \
                 tc.tile_pool(name="psG", bufs=1, space="PSUM") as psG, \
                 tc.tile_pool(name="psT", bufs=2, space="PSUM") as psT:
                qtT = ap.tile([128, 2, BLOC], f16)      # query, [h, b]
                qtT32 = ap.tile([128, 2, BLOC], f32)
                ct = ap.tile([128, 2, BLOC], f32)       # cell state
                att_sb = ap.tile([BLOC, E], f32)
                attT = ap.tile([128, 2, BLOC], f16)
                w1T = ap.tile([128, sum_chunks, BLOC], f16)  # exp weights
                nc.vector.memset(qtT[:], 0.0)
                nc.vector.memset(ct[:], 0.0)

                n_grp = (sum_chunks + 7) // 8
                for it in range(N_SHUFFLE):
                    if it > 0:
                        # logits token-major, chunk-stationary matmuls;
                        # mask-add + exp per 8-chunk group (pipelined)
                        for g in range(n_grp):
                            nch = min(8, sum_chunks - g * 8)
                            lgp = psL.tile([128, 8, BLOC], f32, tag="lgp")
                            for ci in range(nch):
                                c = g * 8 + ci
                                for kc in range(2):
                                    nc.tensor.matmul(
                                        lgp[:, ci, :],
                                        embA[:, kc, c * 128:(c + 1) * 128],
                                        qtT[:, kc, :],
                                        start=(kc == 0), stop=(kc == 1),
                                    )
                            lgs = ad.tile([128, 8, BLOC], f32, tag="lgs")
                            nc.vector.tensor_tensor(
                                out=lgs[:, :nch, :],
                                in0=lgp[:, :nch, :],
                                in1=mask[:, g * 8: g * 8 + nch, :],
                                op=mybir.AluOpType.add,
                            )
                            nc.scalar.activation(
                                out=w1T[:, g * 8: g * 8 + nch, :],
                                in_=lgs[:, :nch, :], func=Exp,
                            )
                        wsrc = w1T
                    else:
                        wsrc = w0T

                    # attended + denominator: accumulate all live chunks
                    att_ps = psA.tile([BLOC, E + 1], f32)
                    for c in range(sum_chunks):
                        nc.tensor.matmul(
                            att_ps[:, :], wsrc[:, c, :], embB[:, c, :],
                            start=(c == 0), stop=(c == sum_chunks - 1),
                        )
                    # normalize: att = att_unnorm * (1/S), S in column E
                    rS = ad.tile([BLOC, 1], f32, tag="rS")
                    nc.vector.reciprocal(rS[:], att_ps[:, E:E + 1])
                    nc.vector.tensor_scalar(
                        out=att_sb[:], in0=att_ps[:, 0:E], scalar1=rS[:],
                        scalar2=None, op0=mybir.AluOpType.mult,
                    )
                    # attT: [16, 256] -> [128, 2, 16]
                    for h in range(2):
                        pt = psT.tile([128, BLOC], f32, tag="pt")
                        nc.tensor.transpose(
                            pt[:], att_sb[:, h * 128:(h + 1) * 128],
                            idf32[:BLOC, :BLOC],
                        )
                        nc.scalar.copy(out=attT[:, h, :], in_=pt[:])

                    # LSTM gates = bg (K=1 matmul) + Wih @ att + Whh @ qt
                    # gate-chunk layout (host-permuted): 0-1=i, 2-3=f,
                    # 4-5=o, 6-7=g
                    g_ps = psG.tile([128, 8, BLOC], f32)
                    for mc in range(8):
                        msl = slice(mc * 128, (mc + 1) * 128)
                        nc.tensor.matmul(
                            g_ps[:, mc, :], bgr[:, msl], onesB[:],
                            start=True, stop=False,
                        )
                        for kc in range(2):
                            nc.tensor.matmul(
                                g_ps[:, mc, :], wih[:, kc, msl], attT[:, kc, :],
                                start=False, stop=False,
                            )
                        for kc in range(2):
                            nc.tensor.matmul(
                                g_ps[:, mc, :], whh[:, kc, msl],
                                qtT[:, kc, :],
                                start=False, stop=(kc == 1),
                            )
                    # sigmoid/tanh via Exp only (no activation-table swap):
                    # sigmoid(x) = 1/(1+e^-x); tanh(x) = 2/(1+e^-2x) - 1
                    eio = ad.tile([128, 6, BLOC], f32, tag="eio")
                    sio = ad.tile([128, 6, BLOC], f32, tag="sio")
                    nc.scalar.activation(
                        out=eio[:], in_=g_ps[:, 0:6, :], func=Exp, scale=-1.0
                    )
                    nc.vector.tensor_scalar(
                        out=eio[:], in0=eio[:], scalar1=1.0, scalar2=None,
                        op0=mybir.AluOpType.add,
                    )
                    nc.vector.reciprocal(sio[:], eio[:])
                    eg = ad.tile([128, 2, BLOC], f32, tag="eg")
                    tg = ad.tile([128, 2, BLOC], f32, tag="tg")
                    nc.scalar.activation(
                        out=eg[:], in_=g_ps[:, 6:8, :], func=Exp, scale=-2.0
                    )
                    nc.vector.tensor_scalar(
                        out=eg[:], in0=eg[:], scalar1=1.0, scalar2=None,
                        op0=mybir.AluOpType.add,
                    )
                    nc.vector.reciprocal(tg[:], eg[:])
                    nc.vector.tensor_scalar(
                        out=tg[:], in0=tg[:], scalar1=2.0, scalar2=-1.0,
                        op0=mybir.AluOpType.mult, op1=mybir.AluOpType.add,
                    )
                    # ct = f*ct + i*g
                    tmp = ad.tile([128, 2, BLOC], f32, tag="tmp")
                    nc.vector.tensor_tensor(
                        out=tmp[:], in0=sio[:, 0:2, :], in1=tg[:],
                        op=mybir.AluOpType.mult,
                    )
                    nc.vector.tensor_tensor(
                        out=ct[:], in0=sio[:, 2:4, :], in1=ct[:],
                        op=mybir.AluOpType.mult,
                    )
                    nc.vector.tensor_tensor(
                        out=ct[:], in0=ct[:], in1=tmp[:], op=mybir.AluOpType.add
                    )
                    # qt = o * tanh(ct)
                    ec = ad.tile([128, 2, BLOC], f32, tag="ec")
                    th = ad.tile([128, 2, BLOC], f32, tag="th")
                    nc.scalar.activation(
                        out=ec[:], in_=ct[:], func=Exp, scale=-2.0
                    )
                    nc.vector.tensor_scalar(
                        out=ec[:], in0=ec[:], scalar1=1.0, scalar2=None,
                        op0=mybir.AluOpType.add,
                    )
                    nc.vector.reciprocal(th[:], ec[:])
                    nc.vector.tensor_scalar(
                        out=th[:], in0=th[:], scalar1=2.0, scalar2=-1.0,
                        op0=mybir.AluOpType.mult, op1=mybir.AluOpType.add,
                    )
                    nc.vector.tensor_tensor(
                        out=qtT32[:], in0=sio[:, 4:6, :], in1=th[:],
                        op=mybir.AluOpType.mult,
                    )
                    nc.vector.tensor_copy(qtT[:], qtT32[:])

                # outputs
                nc.sync.dma_start(out=att_o[:], in_=att_sb[:])
                qt_out = ap.tile([BLOC, H], f32)
                for h in range(2):
                    pt = psT.tile([BLOC, 128], f32, tag="ptq")
                    nc.tensor.transpose(pt[:], qtT32[:, h, :], idf32[:])
                    nc.vector.tensor_copy(qt_out[:, h * 128:(h + 1) * 128], pt[:])
                nc.sync.dma_start(out=qt_o[:], in_=qt_out[:])

    _split_multi_waits(nc)
    return nc


def kernel(state, length, W1, b1, W2, b2, W3, b3, W_ih, W_hh, b_ih, b_hh):
    state = np.asarray(state, dtype=np.float32)
    length = np.asarray(length, dtype=np.int32)
    lengths = length.astype(np.int64)

    # sorted snake assignment: slot j of core c = order[j*8 + c]
    order = np.argsort(-lengths, kind="stable")
    perm = order.reshape(BLOC, NCORES)  # perm[j, c]
    lens = lengths[perm]  # [BLOC, NCORES]
    n_chunks = [int(max(1, int(np.max(np.ceil(lens[j] / 128))))) for j in range(BLOC)]
    sum_chunks = sum(n_chunks)
    t_common = -(-(sum_chunks * 128) // 512) * 512  # pad to 512 multiple
    off = np.concatenate(([0], np.cumsum(np.asarray(n_chunks) * 128)))

    nc = _build_module(n_chunks, t_common)

    # host-side weight prep (shared across cores)
    w1h = W1.T.astype(np.float16)                                # [128, 512]
    w2h = np.ascontiguousarray(
        W2.T.reshape(4, 128, H2).transpose(1, 0, 2)
    ).astype(np.float16)                                         # [128, 4, 512]
    w3h = np.ascontiguousarray(
        W3.T.reshape(4, 128, E).transpose(1, 0, 2)
    ).astype(np.float16)                                         # [128, 4, 256]
    # permute LSTM gate rows from (i,f,g,o) to (i,f,o,g)
    gperm = np.concatenate(
        [np.arange(0, 2 * H), np.arange(3 * H, 4 * H), np.arange(2 * H, 3 * H)]
    )
    Wih_p = W_ih[gperm]
    Whh_p = W_hh[gperm]
    wihh = np.ascontiguousarray(
        Wih_p.T.reshape(2, 128, 4 * H).transpose(1, 0, 2)
    ).astype(np.float16)                                         # [128, 2, 1024]
    whhh = np.ascontiguousarray(
        Whh_p.T.reshape(2, 128, 4 * H).transpose(1, 0, 2)
    ).astype(np.float16)
    b1h = np.ascontiguousarray(b1.reshape(4, 128).T).astype(np.float32)
    b2h = np.ascontiguousarray(b2.reshape(4, 128).T).astype(np.float32)
    # gate bias (b3 folded in: emb tiles lack b3; softmax shift cancels)
    bgv = (b_ih + b_hh + W_ih @ b3)[gperm].astype(np.float32)
    bgrh = bgv[None, :]                                          # [1, 1024]
    idf16h = np.eye(128, dtype=np.float16)
    idf32h = np.eye(128, dtype=np.float32)

    in_maps = []
    for c in range(NCORES):
        bidx = perm[:, c]  # batch index per slot
        xT = np.zeros((128, t_common), dtype=np.float16)
        maskh = np.full((128, sum_chunks, BLOC), NEG, dtype=np.float32)
        w0T = np.zeros((128, sum_chunks, BLOC), dtype=np.float16)
        for j in range(BLOC):
            ln = int(lengths[bidx[j]])
            xT[:, off[j]: off[j] + ln] = state[bidx[j], :ln, :].T
            c0 = off[j] // 128
            valid = np.zeros(n_chunks[j] * 128, dtype=bool)
            valid[:ln] = True
            vT = valid.reshape(n_chunks[j], 128).T  # [128, n_chunks_j]
            maskh[:, c0:c0 + n_chunks[j], j] = np.where(vT, 0.0, NEG)
            w0T[:, c0:c0 + n_chunks[j], j] = np.where(
                vT, 1.0 / ln, 0.0
            ).astype(np.float16)
        in_maps.append({
            "xT": xT, "w1": w1h, "w2": w2h, "w3": w3h,
            "wih": wihh, "whh": whhh, "b1": b1h, "b2": b2h, "bgr": bgrh,
            "mask": maskh, "w0T": w0T, "idf16": idf16h, "idf32": idf32h,
        })

    res = run_bass_kernel_spmd(nc, in_maps, list(range(NCORES)))

    out = np.zeros((B, E + H), dtype=np.float32)
    for c in range(NCORES):
        att = res.results[c]["att"] + b3[None, :].astype(np.float32)
        qt = res.results[c]["qt"]
        for j in range(BLOC):
            out[perm[j, c], :E] = att[j]
            out[perm[j, c], E:] = qt[j]
    return out


# revision 6
# speedup vs baseline: 1.6142x; 1.3514x over previous
"""Trainium2 Bass kernel for nn_CopiedSetEncoder (set encoder with recurrent
attention). Self-contained: shards batch across 8 NeuronCores, builds a
length-specialized SPMD Tile kernel, runs it, and reassembles the output.

v5: fp8(e4m3) attention weights/embB + DoubleRow attended; DoubleRow fp8
W2 layer (power-of-2 scaled, 1/16 folded into f16 W3); softmax
denominator folded into the attended matmul via a ones-column; LSTM
sigmoid/tanh from Exp only (no activation-table swaps) with gate biases
folded in as precomputed exp(-b) broadcast multiplies; staged input DMA.
"""
import os

import numpy as np

import concourse.bass as bass
import concourse.mybir as mybir
import concourse.tile as tile
from concourse.bass_utils import run_bass_kernel_spmd

B, F_, D_IN = 128, 1024, 128
H1, H2, E, H = 512, 512, 256, 256
N_SHUFFLE = 5
NCORES = 8
BLOC = B // NCORES  # 16 batches per core
NEG = -1e30

f32 = mybir.dt.float32
f16 = mybir.dt.float16
f8 = mybir.dt.float8e4


def _split_multi_waits(nc):
    """HW allows at most one sync wait per instruction; hoist extras into
    standalone InstEventSemaphore carriers on the same engine."""
    cnt = 0
    for bb in nc.main_func.blocks:
        insts = bb.instructions  # live list
        i = 0
        while i < len(insts):
            ins = insts[i]
            si = ins.sync_info
            if si is not None and si.on_wait and len(si.on_wait) > 1:
                waits = list(si.on_wait)
                carriers = []
                for w in waits[:-1]:
                    cnt += 1
                    ev = mybir.InstEventSemaphore(name=f"wsplit-{cnt}")
                    ev.engine = ins.engine
                    ev.sync_info = mybir.SyncInfo(on_wait=[w], on_update=[])
                    carriers.append(ev)
                ins.sync_info = mybir.SyncInfo(
                    on_wait=[waits[-1]], on_update=list(si.on_update)
                )
                for j, ev in enumerate(carriers):
                    insts.insert(i + j, ev)
                    nc.register_instruction(ev, overwrite=True)
                i += len(carriers)
            i += 1
    return cnt


PHASE = os.environ.get("KER_PHASE", "full")
FP8W2 = os.environ.get("KER_FP8W2", "1") == "1"
FP8ATT = os.environ.get("KER_FP8ATT", "1") == "1"
Exp = mybir.ActivationFunctionType.Exp
Relu = mybir.ActivationFunctionType.Relu
DR = mybir.MatmulPerfMode.DoubleRow
W2S = 16.0  # power-of-2 scale keeps fp8 W2 out of subnormals; 1/W2S folds into f16 W3


def _build_module(n_chunks, t_common):
    """One SPMD program for all cores. n_chunks[j] = 128-token chunks for
    batch slot j (uniform across cores); t_common = sum(n_chunks)*128
    rounded up to a 512 multiple."""
    nc = bass.Bass()
    n_tiles = t_common // 512
    tot_chunks = t_common // 128
    sum_chunks = sum(n_chunks)  # live chunks (tail padding excluded)
    tot_even = sum_chunks + (sum_chunks & 1)

    dB = f8 if FP8ATT else f16       # embB / attention-weight element type
    dM = f8 if FP8W2 else f16        # h1/W2 element type
    EB = 272 if FP8ATT else 257      # embB row: E + ones col (+pad to 16)
    w0w = tot_even if FP8ATT else sum_chunks

    # ---- inputs ----
    xT_e = nc.declare_dram_parameter("xT", [128, t_common], f16, isOutput=False)
    w1_e = nc.declare_dram_parameter("w1", [128, H1], f16, isOutput=False)
    w2_e = nc.declare_dram_parameter("w2", [128, 4, H2], dM, isOutput=False)
    w3_e = nc.declare_dram_parameter("w3", [128, 4, E], f16, isOutput=False)
    wih_e = nc.declare_dram_parameter("wih", [128, 2, 4 * H], f16, isOutput=False)
    whh_e = nc.declare_dram_parameter("whh", [128, 2, 4 * H], f16, isOutput=False)
    b1_e = nc.declare_dram_parameter("b1", [128, 4], f32, isOutput=False)
    b2_e = nc.declare_dram_parameter("b2", [128, 4], f32, isOutput=False)
    ebg_e = nc.declare_dram_parameter("ebg", [128, 8], f32, isOutput=False)
    mask_e = nc.declare_dram_parameter(
        "mask", [128, sum_chunks, BLOC], f32, isOutput=False
    )
    w0T_e = nc.declare_dram_parameter("w0T", [128, w0w, BLOC], dB, isOutput=False)
    idf16_e = nc.declare_dram_parameter("idf16", [128, 128], f16, isOutput=False)
    idf32_e = nc.declare_dram_parameter("idf32", [128, 128], f32, isOutput=False)
    att_o = nc.declare_dram_parameter("att", [BLOC, E], f32, isOutput=True)
    qt_o = nc.declare_dram_parameter("qt", [BLOC, H], f32, isOutput=True)

    with tile.TileContext(nc) as tc:
        with tc.tile_pool(name="big", bufs=1) as big, \
             tc.tile_pool(name="wp", bufs=1) as wp:
            # resident tensors
            xT = big.tile([128, t_common], f16)
            embA = big.tile([128, 2, t_common], f16)
            embB = big.tile([128, tot_chunks, EB], dB)
            w1 = wp.tile([128, H1], f16)
            w2 = wp.tile([128, 4, H2], dM)
            w3 = wp.tile([128, 4, E], f16)
            wih = wp.tile([128, 2, 4 * H], f16)
            whh = wp.tile([128, 2, 4 * H], f16)
            b1 = wp.tile([128, 4], f32)
            b2 = wp.tile([128, 4], f32)
            ebg = wp.tile([128, 8], f32)
            mask = wp.tile([128, sum_chunks, BLOC], f32)
            w0T = wp.tile([128, w0w, BLOC], dB)
            idf16 = wp.tile([128, 128], f16)
            idf32 = wp.tile([128, 128], f32)

            # staged DMA: phase-1-critical first, phase-2 tensors later
            nc.sync.dma_start(out=w1[:], in_=w1_e[:])
            nc.sync.dma_start(out=b1[:], in_=b1_e[:])
            # xT in pieces so tile-0 compute starts immediately
            xsplit = sorted({min(v, t_common) for v in (0, 512, 1024, 3072, 6144)})
            xsplit.append(t_common)
            for a, b in zip(xsplit[:-1], xsplit[1:]):
                if b > a:
                    nc.sync.dma_start(out=xT[:, a:b], in_=xT_e[:, a:b])
            nc.sync.dma_start(out=w2[:], in_=w2_e[:])
            nc.sync.dma_start(out=b2[:], in_=b2_e[:])
            nc.sync.dma_start(out=w3[:], in_=w3_e[:])
            nc.sync.dma_start(out=idf16[:], in_=idf16_e[:])
            nc.sync.dma_start(out=idf32[:], in_=idf32_e[:])
            nc.sync.dma_start(out=w0T[:], in_=w0T_e[:])
            nc.sync.dma_start(out=wih[:], in_=wih_e[:])
            nc.sync.dma_start(out=whh[:], in_=whh_e[:])
            nc.sync.dma_start(out=ebg[:], in_=ebg_e[:])
            nc.sync.dma_start(out=mask[:], in_=mask_e[:])
            # softmax-denominator column of embB (mask already zeroes the
            # weights of invalid tokens, so a plain 1 everywhere is exact)
            nc.vector.memset(embB[:, :, E], 1.0)
            if EB > E + 1:
                nc.vector.memset(embB[:, :, E + 1:EB], 0.0)

            # ---- phase 1: MLP over 512-token tiles; embB by transpose ----
            with tc.tile_pool(name="mlp", bufs=3) as mp, \
                 tc.tile_pool(name="ps1", bufs=2, space="PSUM") as ps1, \
                 tc.tile_pool(name="ps2", bufs=2, space="PSUM") as ps2, \
                 tc.tile_pool(name="ps3", bufs=2, space="PSUM") as ps3, \
                 tc.tile_pool(name="psTr", bufs=2, space="PSUM") as psTr:
                for t in range(n_tiles):
                    sl = slice(t * 512, (t + 1) * 512)
                    h1t = mp.tile([128, 4, 512], dM, tag="h1")
                    for mc in range(4):
                        p = ps1.tile([128, 512], f32, tag="pA")
                        nc.tensor.matmul(
                            p[:], w1[:, mc * 128:(mc + 1) * 128], xT[:, sl],
                            start=True, stop=True,
                        )
                        if mc % 2 == 0:
                            nc.scalar.activation(
                                out=h1t[:, mc, :], in_=p[:], func=Relu,
                                bias=b1[:, mc:mc + 1], scale=1.0,
                            )
                        else:
                            nc.vector.tensor_scalar(
                                out=h1t[:, mc, :], in0=p[:], scalar1=b1[:, mc:mc + 1],
                                scalar2=0.0, op0=mybir.AluOpType.add,
                                op1=mybir.AluOpType.max,
                            )
                    # h2' = relu(W2S*W2 @ h1 + W2S*b2); stored scaled by W2S
                    h2t = mp.tile([128, 4, 512], f16, tag="h2")
                    for mc in range(4):
                        p = ps2.tile([128, 512], f32, tag="pB")
                        if FP8W2:
                            for kp in range(2):
                                nc.tensor.matmul(
                                    p[:],
                                    w2[:, 2 * kp:2 * kp + 2, mc * 128:(mc + 1) * 128],
                                    h1t[:, 2 * kp:2 * kp + 2, :],
                                    start=(kp == 0), stop=(kp == 1),
                                    perf_mode=DR,
                                )
                        else:
                            for kc in range(4):
                                nc.tensor.matmul(
                                    p[:], w2[:, kc, mc * 128:(mc + 1) * 128],
                                    h1t[:, kc, :], start=(kc == 0), stop=(kc == 3),
                                )
                        if mc % 2 == 0:
                            nc.scalar.activation(
                                out=h2t[:, mc, :], in_=p[:], func=Relu,
                                bias=b2[:, mc:mc + 1], scale=1.0,
                            )
                        else:
                            nc.vector.tensor_scalar(
                                out=h2t[:, mc, :], in0=p[:], scalar1=b2[:, mc:mc + 1],
                                scalar2=0.0, op0=mybir.AluOpType.add,
                                op1=mybir.AluOpType.max,
                            )
                    # embA = (W3/W2S) @ h2' (h2' carries the W2S factor)
                    for mc in range(2):
                        p = ps3.tile([128, 512], f32, tag="pC")
                        for kc in range(4):
                            nc.tensor.matmul(
                                p[:], w3[:, kc, mc * 128:(mc + 1) * 128],
                                h2t[:, kc, :], start=(kc == 0), stop=(kc == 3),
                            )
                        nc.scalar.copy(out=embA[:, mc, sl], in_=p[:])
                    # embB: [token partitions, e] via PE transposes of embA
                    for s in range(4):
                        c = t * 4 + s
                        tsl = slice(t * 512 + s * 128, t * 512 + (s + 1) * 128)
                        for h in range(2):
                            pt = psTr.tile([128, 128], f16, tag="pT")
                            nc.tensor.transpose(pt[:], embA[:, h, tsl], idf16[:])
                            nc.vector.tensor_copy(
                                embB[:, c, h * 128:(h + 1) * 128], pt[:]
                            )

            if PHASE == "mlp":
                with tc.tile_pool(name="dummy", bufs=1) as dp:
                    da = dp.tile([BLOC, E], f32)
                    dq = dp.tile([BLOC, H], f32)
                    nc.vector.tensor_copy(da[:], embA[:BLOC, 0, :E])
                    nc.vector.tensor_copy(dq[:], embB[:BLOC, 0, :H])
                    nc.sync.dma_start(out=att_o[:], in_=da[:])
                    nc.sync.dma_start(out=qt_o[:], in_=dq[:])

            # ---- phase 2: recurrent attention ----
            if PHASE != "mlp":
              with tc.tile_pool(name="att", bufs=1) as ap, \
                 tc.tile_pool(name="attd", bufs=2) as ad, \
                 tc.tile_pool(name="psL", bufs=2, space="PSUM") as psL, \
                 tc.tile_pool(name="psA", bufs=1, space="PSUM") as psA, \
                 tc.tile_pool(name="psG", bufs=1, space="PSUM") as psG, \
                 tc.tile_pool(name="psT", bufs=2, space="PSUM") as psT:
                qtT = ap.tile([128, 2, BLOC], f16)      # query, [h, b]
                ct = ap.tile([128, 2, BLOC], f32)       # cell state
                att_sb = ap.tile([BLOC, E], f32)
                attT = ap.tile([128, 2, BLOC], f16)
                w1T = ap.tile([128, w0w, BLOC], dB)     # exp weights
                nc.vector.memset(qtT[:], 0.0)
                nc.vector.memset(ct[:], 0.0)
                if w0w > sum_chunks:
                    nc.vector.memset(w1T[:, sum_chunks:, :], 0.0)

                # broadcast APs for the per-gate-chunk bias factors
                eb1 = ebg[:, 0:6]
                ebio_b = bass.AP(
                    tensor=eb1.tensor, offset=eb1.offset,
                    ap=[list(eb1.ap[0]), list(eb1.ap[1]), [0, BLOC]],
                )
                eb2 = ebg[:, 6:8]
                ebg_b = bass.AP(
                    tensor=eb2.tensor, offset=eb2.offset,
                    ap=[list(eb2.ap[0]), list(eb2.ap[1]), [0, BLOC]],
                )

                n_grp = (sum_chunks + 7) // 8
                for it in range(N_SHUFFLE):
                    if it > 0:
                        # logits token-major, chunk-stationary matmuls;
                        # mask-add + exp per 8-chunk group (pipelined)
                        for g in range(n_grp):
                            nch = min(8, sum_chunks - g * 8)
                            lgp = psL.tile([128, 8, BLOC], f32, tag="lgp")
                            for ci in range(nch):
                                c = g * 8 + ci
                                for kc in range(2):
                                    nc.tensor.matmul(
                                        lgp[:, ci, :],
                                        embA[:, kc, c * 128:(c + 1) * 128],
                                        qtT[:, kc, :],
                                        start=(kc == 0), stop=(kc == 1),
                                    )
                            lgs = ad.tile([128, 8, BLOC], f32, tag="lgs")
                            nc.vector.tensor_tensor(
                                out=lgs[:, :nch, :],
                                in0=lgp[:, :nch, :],
                                in1=mask[:, g * 8: g * 8 + nch, :],
                                op=mybir.AluOpType.add,
                            )
                            nc.scalar.activation(
                                out=w1T[:, g * 8: g * 8 + nch, :],
                                in_=lgs[:, :nch, :], func=Exp,
                            )
                        wsrc = w1T
                    else:
                        wsrc = w0T

                    # attended + denominator over all live chunks
                    att_ps = psA.tile([BLOC, EB], f32)
                    if FP8ATT:
                        npair = tot_even // 2
                        for q in range(npair):
                            nc.tensor.matmul(
                                att_ps[:, :], wsrc[:, 2 * q:2 * q + 2, :],
                                embB[:, 2 * q:2 * q + 2, :],
                                start=(q == 0), stop=(q == npair - 1),
                                perf_mode=DR,
                            )
                    else:
                        for c in range(sum_chunks):
                            nc.tensor.matmul(
                                att_ps[:, :], wsrc[:, c, :], embB[:, c, :],
                                start=(c == 0), stop=(c == sum_chunks - 1),
                            )
                    # normalize: att = att_unnorm * (1/S), S in column E
                    rS = ad.tile([BLOC, 1], f32, tag="rS")
                    nc.vector.reciprocal(rS[:], att_ps[:, E:E + 1])
                    nc.vector.tensor_scalar(
                        out=att_sb[:], in0=att_ps[:, 0:E], scalar1=rS[:],
                        scalar2=None, op0=mybir.AluOpType.mult,
                    )
                    # attT: [16, 256] -> [128, 2, 16]
                    for h in range(2):
                        pt = psT.tile([128, BLOC], f32, tag="pt")
                        nc.tensor.transpose(
                            pt[:], att_sb[:, h * 128:(h + 1) * 128],
                            idf32[:BLOC, :BLOC],
                        )
                        nc.scalar.copy(out=attT[:, h, :], in_=pt[:])

                    # LSTM gates = Wih @ att + Whh @ qt (bias folded into
                    # the exp factors below); host gate-chunk layout:
                    # 0-1=i, 2-3=f, 4-5=o, 6-7=g
                    g_ps = psG.tile([128, 8, BLOC], f32)
                    for mc in range(8):
                        msl = slice(mc * 128, (mc + 1) * 128)
                        for kc in range(2):
                            nc.tensor.matmul(
                                g_ps[:, mc, :], wih[:, kc, msl], attT[:, kc, :],
                                start=(kc == 0), stop=False,
                            )
                        for kc in range(2):
                            nc.tensor.matmul(
                                g_ps[:, mc, :], whh[:, kc, msl],
                                qtT[:, kc, :],
                                start=False, stop=(kc == 1),
                            )
                    # sigmoid/tanh via Exp only (no activation-table swap):
                    # sigmoid(x+b) = 1/(1+e^-x * e^-b)
                    eio = ad.tile([128, 6, BLOC], f32, tag="eio")
                    sio = ad.tile([128, 6, BLOC], f32, tag="sio")
                    nc.scalar.activation(
                        out=eio[:], in_=g_ps[:, 0:6, :], func=Exp, scale=-1.0
                    )
                    nc.vector.tensor_tensor(
                        out=eio[:], in0=eio[:], in1=ebio_b,
                        op=mybir.AluOpType.mult,
                    )
                    nc.vector.tensor_scalar(
                        out=eio[:], in0=eio[:], scalar1=1.0, scalar2=None,
                        op0=mybir.AluOpType.add,
                    )
                    nc.vector.reciprocal(sio[:], eio[:])
                    # tanh(x+b) = 2/(1+e^-2x * e^-2b) - 1
                    eg = ad.tile([128, 2, BLOC], f32, tag="eg")
                    tg = ad.tile([128, 2, BLOC], f32, tag="tg")
                    nc.scalar.activation(
                        out=eg[:], in_=g_ps[:, 6:8, :], func=Exp, scale=-2.0
                    )
                    nc.vector.tensor_tensor(
                        out=eg[:], in0=eg[:], in1=ebg_b, op=mybir.AluOpType.mult
                    )
                    nc.vector.tensor_scalar(
                        out=eg[:], in0=eg[:], scalar1=1.0, scalar2=None,
                        op0=mybir.AluOpType.add,
                    )
                    nc.vector.reciprocal(tg[:], eg[:])
                    nc.vector.tensor_scalar(
                        out=tg[:], in0=tg[:], scalar1=2.0, scalar2=-1.0,
                        op0=mybir.AluOpType.mult, op1=mybir.AluOpType.add,
                    )
                    # ct = f*ct + i*g
                    tmp = ad.tile([128, 2, BLOC], f32, tag="tmp")
                    nc.vector.tensor_tensor(
                        out=tmp[:], in0=sio[:, 0:2, :], in1=tg[:],
                        op=mybir.AluOpType.mult,
                    )
                    nc.vector.tensor_tensor(
                        out=ct[:], in0=sio[:, 2:4, :], in1=ct[:],
                        op=mybir.AluOpType.mult,
                    )
                    nc.vector.tensor_tensor(
                        out=ct[:], in0=ct[:], in1=tmp[:], op=mybir.AluOpType.add
                    )
                    # qt = o * tanh(ct)
                    ec = ad.tile([128, 2, BLOC], f32, tag="ec")
                    th = ad.tile([128, 2, BLOC], f32, tag="th")
                    nc.scalar.activation(
                        out=ec[:], in_=ct[:], func=Exp, scale=-2.0
                    )
                    nc.vector.tensor_scalar(
                        out=ec[:], in0=ec[:], scalar1=1.0, scalar2=None,
                        op0=mybir.AluOpType.add,
                    )
                    nc.vector.reciprocal(th[:], ec[:])
                    nc.vector.tensor_scalar(
                        out=th[:], in0=th[:], scalar1=2.0, scalar2=-1.0,
                        op0=mybir.AluOpType.mult, op1=mybir.AluOpType.add,
                    )
                    nc.vector.tensor_tensor(
                        out=qtT[:], in0=sio[:, 4:6, :], in1=th[:],
                        op=mybir.AluOpType.mult,
                    )

                # outputs
                nc.sync.dma_start(out=att_o[:], in_=att_sb[:])
                qt_out = ap.tile([BLOC, H], f32)
                for h in range(2):
                    pt = psT.tile([BLOC, 128], f16, tag="ptq")
                    nc.tensor.transpose(pt[:], qtT[:, h, :], idf16[:])
                    nc.vector.tensor_copy(qt_out[:, h * 128:(h + 1) * 128], pt[:])
                nc.sync.dma_start(out=qt_o[:], in_=qt_out[:])

    _split_multi_waits(nc)
    return nc


def kernel(state, length, W1, b1, W2, b2, W3, b3, W_ih, W_hh, b_ih, b_hh):
    state = np.asarray(state, dtype=np.float32)
    length = np.asarray(length, dtype=np.int32)
    lengths = length.astype(np.int64)

    # sorted snake assignment: slot j of core c = order[j*8 + c]
    order = np.argsort(-lengths, kind="stable")
    perm = order.reshape(BLOC, NCORES)  # perm[j, c]
    lens = lengths[perm]  # [BLOC, NCORES]
    n_chunks = [int(max(1, int(np.max(np.ceil(lens[j] / 128))))) for j in range(BLOC)]
    sum_chunks = sum(n_chunks)
    tot_even = sum_chunks + (sum_chunks & 1)
    w0w = tot_even if FP8ATT else sum_chunks
    t_common = -(-(sum_chunks * 128) // 512) * 512  # pad to 512 multiple
    off = np.concatenate(([0], np.cumsum(np.asarray(n_chunks) * 128)))

    nc = _build_module(n_chunks, t_common)

    np8 = mybir.dt.np(f8)
    dBn = np8 if FP8ATT else np.float16
    dMn = np8 if FP8W2 else np.float16

    # host-side weight prep (shared across cores)
    w1h = W1.T.astype(np.float16)                                # [128, 512]
    w2s = W2S if FP8W2 else 1.0
    w2h = np.ascontiguousarray(
        (W2 * w2s).T.reshape(4, 128, H2).transpose(1, 0, 2)
    ).astype(dMn)                                                # [128, 4, 512]
    w3h = np.ascontiguousarray(
        (W3 / w2s).T.reshape(4, 128, E).transpose(1, 0, 2)
    ).astype(np.float16)                                         # [128, 4, 256]
    # permute LSTM gate rows from (i,f,g,o) to (i,f,o,g)
    gperm = np.concatenate(
        [np.arange(0, 2 * H), np.arange(3 * H, 4 * H), np.arange(2 * H, 3 * H)]
    )
    Wih_p = W_ih[gperm]
    Whh_p = W_hh[gperm]
    wihh = np.ascontiguousarray(
        Wih_p.T.reshape(2, 128, 4 * H).transpose(1, 0, 2)
    ).astype(np.float16)                                         # [128, 2, 1024]
    whhh = np.ascontiguousarray(
        Whh_p.T.reshape(2, 128, 4 * H).transpose(1, 0, 2)
    ).astype(np.float16)
    b1h = np.ascontiguousarray(b1.reshape(4, 128).T).astype(np.float32)
    b2h = np.ascontiguousarray(
        (b2 * w2s).reshape(4, 128).T
    ).astype(np.float32)
    # gate bias (b3 folded in: emb tiles lack b3; softmax shift cancels),
    # applied as multiplicative exp factors inside the Exp-only LSTM
    bgv = (b_ih + b_hh + W_ih @ b3)[gperm].astype(np.float64)
    bgc = bgv.reshape(8, 128).T                                  # [128, 8]
    ebgh = np.empty((128, 8), np.float32)
    ebgh[:, 0:6] = np.exp(-bgc[:, 0:6])
    ebgh[:, 6:8] = np.exp(-2.0 * bgc[:, 6:8])
    idf16h = np.eye(128, dtype=np.float16)
    idf32h = np.eye(128, dtype=np.float32)

    w0scale = 64.0 if FP8ATT else 1.0  # keep 1/len out of fp8 subnormals

    in_maps = []
    for c in range(NCORES):
        bidx = perm[:, c]  # batch index per slot
        xT = np.zeros((128, t_common), dtype=np.float16)
        maskh = np.full((128, sum_chunks, BLOC), NEG, dtype=np.float32)
        w0T = np.zeros((128, w0w, BLOC), dtype=dBn)
        for j in range(BLOC):
            ln = int(lengths[bidx[j]])
            xT[:, off[j]: off[j] + ln] = state[bidx[j], :ln, :].T
            c0 = off[j] // 128
            valid = np.zeros(n_chunks[j] * 128, dtype=bool)
            valid[:ln] = True
            vT = valid.reshape(n_chunks[j], 128).T  # [128, n_chunks_j]
            maskh[:, c0:c0 + n_chunks[j], j] = np.where(vT, 0.0, NEG)
            w0T[:, c0:c0 + n_chunks[j], j] = np.where(
                vT, w0scale / ln, 0.0
            ).astype(dBn)
        in_maps.append({
            "xT": xT, "w1": w1h, "w2": w2h, "w3": w3h,
            "wih": wihh, "whh": whhh, "b1": b1h, "b2": b2h, "ebg": ebgh,
            "mask": maskh, "w0T": w0T, "idf16": idf16h,
            "idf32": idf32h,
        })

    res = run_bass_kernel_spmd(nc, in_maps, list(range(NCORES)))

    out = np.zeros((B, E + H), dtype=np.float32)
    for c in range(NCORES):
        att = res.results[c]["att"] + b3[None, :].astype(np.float32)
        qt = res.results[c]["qt"]
        for j in range(BLOC):
            out[perm[j, c], :E] = att[j]
            out[perm[j, c], E:] = qt[j]
    return out
